# revision 9
# baseline (speedup 1.0000x reference)
"""Trainium2 Bass kernel for nn_MDSFF (deformable-sampling sparse attention).

Math restructuring (key to making this fast on TRN2):
  - Offsets are tanh-bounded to +-1 px, so bilinear grid-sample == 9-tap
    local stencil with per-pixel weights w_d = relu(1 - |clip(t,lo,hi) - d|),
    zeroed at image edges.
  - 1x1 convs commute with spatial shifts: k_proj(sampled) = sum_t tw_t *
    shift_t(k_w @ x_aux), and the output projection folds the same way, so
    the huge [B,K,C,H,W] sampled tensor is never materialized.
  - sim[k,h] = sum_t tw[k,t] * S_t[h], where S_t[h] = sum_{d in head h}
    q * shift_t(A); S_t is computed once per tap (not per k).
  - final = sum_t G_t * shift_t(out_w @ x_aux), G_t = sum_k tw[k,t]*wk[k].
  - tw never materialized: (wy*wx)*X computed as wy*(wx*X).

Sharding: 8 cores = 4 batches x 2 H-halves. Each core processes its half in
two 16-row column chunks. Host (numpy) does only data movement: slicing,
halo padding, layout, constant masks/selectors.
"""

import sys

sys.path.insert(0, "/opt/trn_rl_repo")

import numpy as np

import concourse.bass as bass
import concourse.mybir as mybir
from concourse import tile
from concourse.bass_utils import run_bass_kernel_spmd

# ---------------- problem constants (hardcoded per contract) ----------------
B, C, H, W = 4, 256, 64, 64
K = 8
NHEADS = 4
NCORES = 8
ROWS = 32         # center rows per core
CHR = 16          # rows per chunk
NCHUNK = 2
N1 = CHR * W      # 1024 center pixels per chunk
HR = 34           # haloed rows per core
XA_W = HR * W     # 2176
XM_W = 2248       # padded x_main width: col = 2 + 66*r + w
AW = 2 + 18 * W   # 1154: per-chunk A/Ao width, data cols [1, 1153)
TAPS = [(dy, dx) for dy in (-1, 0, 1) for dx in (-1, 0, 1)]

F32 = mybir.dt.float32
R32 = mybir.dt.float32r
AF = mybir.ActivationFunctionType
OP = mybir.AluOpType

USE_F32R = False   # matmul operands as float32r (1 cyc/row vs 4 for fp32)

_CACHE = {}


def _mmcast(ap):
    return ap.bitcast(R32) if USE_F32R else ap


# ============================ program builder ===============================

def _build_program():
    MAX_WAITS = 1

    SPLIT_OK = {
        "InstDrain", "InstNoOp", "InstMatmult", "InstLdweights",
        "InstTensorTensor", "InstActivation", "InstTensorScalarPtr",
        "InstTensorReduce", "InstCopy", "InstMemSet", "InstReciprocal",
        "InstTensorTensorReduce", "InstTensorCopy",
    }

    def split_waits(nc):
        # walrus in this container rejects instructions carrying more than
        # MAX_WAITS semaphore waits; spill extras onto same-engine nops.
        # Only safe for engine-FIFO instructions: hoisting a DMA descriptor's
        # wait onto the SP sequencer can deadlock (SP stalls instead of the
        # DGE queue, while producers wait on later SP-pushed DMAs).
        f = nc.m.functions[0]
        for bb in f.blocks:
            insts = bb.instructions
            out = []
            changed = False
            for inst in insts:
                si = inst.sync_info
                waits = list(si.on_wait) if si and si.on_wait else []
                if (len(waits) > MAX_WAITS
                        and type(inst).__name__ in SPLIT_OK
                        and all(w.wait_reg is None for w in waits)):
                    changed = True
                    rest, keep = waits[:-MAX_WAITS], waits[-MAX_WAITS:]
                    for i in range(0, len(rest), MAX_WAITS):
                        nop = mybir.InstNoOp(
                            name=f"Wspill_{inst.name}_{i}", ins=[], outs=[])
                        nop.engine = inst.engine
                        nop.sync_info = mybir.SyncInfo(
                            on_wait=rest[i : i + MAX_WAITS], on_update=[])
                        nc.register_instruction(nop)
                        out.append(nop)
                    inst.sync_info = mybir.SyncInfo(
                        on_wait=keep, on_update=list(si.on_update or [])
                    )
                out.append(inst)
            if changed:
                bb.instructions = out

    nc = bass.Bass("TRN2", target_bir_lowering=False, debug=False,
                   num_devices=NCORES)

    dp = nc.dram_tensor
    xm_d = dp("xm", [128, 2, XM_W], F32, kind="ExternalInput")
    xa_d = dp("xa", [128, 2, XA_W], F32, kind="ExternalInput")
    # packed weights: qwT(512) kwT(512) owT(512) cwT(288) sel(64) i128(128)
    wcat_d = dp("wcat", [128, 2016], F32, kind="ExternalInput")
    # packed 32-row consts: i32|hs|avg4|qb (224), sel16 (64), offb (1),
    # dbias (4), lox|hix|mxm|mxp (4*64), loy|hiy|mym|myp (4*32)
    mcat_d = dp("mcat", [32, 677], F32, kind="ExternalInput")
    y_d = dp("y", [128, 2, ROWS * W], F32, kind="ExternalOutput")

    V = nc.vector
    A_ = nc.scalar

    def mm(out, lhsT, rhs, start, stop):
        nc.tensor.matmul(out=out, lhsT=_mmcast(lhsT), rhs=_mmcast(rhs),
                         start=start, stop=stop, skip_group_check=True)

    with tile.TileContext(nc) as tc:
        with (
            tc.tile_pool(name="pw", bufs=1) as pw,          # weights/selectors
            tc.tile_pool(name="pio", bufs=1) as pio,        # xm, xa
            tc.tile_pool(name="pbig", bufs=1) as pbig,      # q, A, Ao, out
            tc.tile_pool(name="pmap", bufs=1) as pmap,      # 32-row maps
            tc.tile_pool(name="pm", bufs=1) as pm,          # big S-stage temps
            tc.tile_pool(name="psum", bufs=1, space="PSUM") as psp,
        ):
            xm = pio.tile([128, 2, XM_W], F32, tag="xm")
            xa = pio.tile([128, 2, XA_W], F32, tag="xa")
            nc.sync.dma_start(out=xm[:], in_=xm_d[:])
            nc.sync.dma_start(out=xa[:], in_=xa_d[:])

            wcat = pw.tile([128, 2016], F32, tag="wcat")
            mcat = pw.tile([32, 677], F32, tag="mcat")
            nc.sync.dma_start(out=wcat[:], in_=wcat_d[:])
            nc.sync.dma_start(out=mcat[:], in_=mcat_d[:])

            def w4(o):  # [128, 2, 2, 128] block at col o
                return wcat[:, o : o + 512].rearrange(
                    "p (cb ob m) -> p cb ob m", cb=2, ob=2)

            qwT, kwT, owT = w4(0), w4(512), w4(1024)
            cwT = wcat[:, 1536:1824].rearrange("p (t cb m) -> p t cb m", t=9, cb=2)
            sel = wcat[:, 1824:1888].rearrange("p (cb j) -> p cb j", cb=2)
            i128 = wcat[:, 1888:2016]
            i32 = mcat[:, 0:32]
            hs = mcat[:, 32:64]
            avg4 = mcat[:, 64:96]
            qb = mcat[:, 96:224]
            sel16 = mcat[0:16, 224:288]
            offb = mcat[0:16, 288:289]
            dbias = mcat[:, 289:293]

            def xconst(i):   # [32, 16, 64] broadcast of a 64-wide column const
                return (mcat[:, None, 293 + 64 * i : 357 + 64 * i]
                        .broadcast_to([32, CHR, 64]))

            def yconst(i, ch):  # [32, 16, 64] broadcast of a per-row const
                c0 = 549 + 32 * i + CHR * ch
                return (mcat[:, c0 : c0 + CHR, None]
                        .broadcast_to([32, CHR, 64]))

            def xm_view(cb, row0, dy=0, dx=0):
                # [128, 8, 64] view of padded x_main: 8 rows starting at
                # haloed local row (1 + row0 + dy), w-shift dx.
                o = 2 + 66 * (1 + row0 + dy) + dx
                return (xm[:, cb, o : o + 8 * 66]
                        .rearrange("p (r w) -> p r w", w=66)[:, :, 0:64])

            for ch in range(NCHUNK):
                R0 = CHR * ch

                # ---------------- offset conv + tanh ----------------
                off = pmap.tile([16, N1], F32, tag="off", name=f"off{ch}")
                for i in range(2):
                    pso = psp.tile([128, 512], F32, tag="ps512", bufs=2,
                                   name=f"pso{ch}{i}")
                    for t in range(9):
                        dy, dx = TAPS[t]
                        for cb in range(2):
                            mm(pso[0:16, :], cwT[:, t, cb, :],
                               xm_view(cb, R0 + 8 * i, dy, dx),
                               start=(t == 0 and cb == 0),
                               stop=(t == 8 and cb == 1))
                    A_.activation(out=off[:, 512 * i : 512 * (i + 1)],
                                  in_=pso[0:16, :], func=AF.Tanh,
                                  bias=offb, scale=1.0)

                # ---------------- q / A / Ao projections ----------------
                q = pbig.tile([128, 2, N1], F32, tag="q", name=f"q{ch}")
                for ob in range(2):
                    for i in range(2):
                        ps = psp.tile([128, 512], F32, tag="ps512", bufs=2,
                                      name=f"psq{ch}{ob}{i}")
                        for cb in range(2):
                            mm(ps[:], qwT[:, cb, ob, :], xm_view(cb, R0 + 8 * i),
                               start=(cb == 0), stop=(cb == 1))
                        A_.activation(out=q[:, ob, 512 * i : 512 * (i + 1)],
                                      in_=ps[:], func=AF.Copy)

                AT = pbig.tile([128, 2, AW], F32, tag="A", name=f"A{ch}")
                AoT = pbig.tile([128, 2, AW], F32, tag="Ao", name=f"Ao{ch}")
                for dst in (AT, AoT):
                    V.memset(dst[:, :, 0:1], 0.0)
                    V.memset(dst[:, :, AW - 1 : AW], 0.0)
                for di, (dst, wT) in enumerate(((AT, kwT), (AoT, owT))):
                    for ob in range(2):
                        for j, sz in ((0, 512), (1, 512), (2, 128)):
                            ps = psp.tile([128, 512], F32, tag="ps512", bufs=2,
                                          name=f"psP{ch}{di}{ob}{j}")
                            rhs = xa[:, :, 64 * R0 + 512 * j : 64 * R0 + 512 * j + sz]
                            for cb in range(2):
                                mm(ps[:, 0:sz], wT[:, cb, ob, :], rhs[:, cb, :],
                                   start=(cb == 0), stop=(cb == 1))
                            A_.activation(
                                out=dst[:, ob, 1 + 512 * j : 1 + 512 * j + sz],
                                in_=ps[:, 0:sz], func=AF.Copy)

                # ---------------- tap-weight maps ----------------
                # T2[:, 0, :] = tx (replicated over heads), T2[:, 1, :] = ty
                T2 = pmap.tile([32, 2, N1], F32, tag="T2", name=f"T2{ch}")
                for i in range(2):
                    ps64 = psp.tile([128, 512], F32, tag="ps512", bufs=2,
                                    name=f"ps64{ch}{i}")
                    mm(ps64[0:64, :], sel16, off[:, 512 * i : 512 * (i + 1)],
                       start=True, stop=True)
                    sl = slice(512 * i, 512 * (i + 1))
                    A_.activation(out=T2[:, 0, sl], in_=ps64[0:32, :], func=AF.Copy)
                    A_.activation(out=T2[:, 1, sl], in_=ps64[32:64, :], func=AF.Copy)

                txv = T2[:, 0, :].rearrange("p (r w) -> p r w", w=64)
                tyv = T2[:, 1, :].rearrange("p (r w) -> p r w", w=64)
                V.tensor_tensor(out=txv, in0=txv, in1=xconst(0), op=OP.max)
                V.tensor_tensor(out=txv, in0=txv, in1=xconst(1), op=OP.min)
                V.tensor_tensor(out=tyv, in0=tyv, in1=yconst(0, ch), op=OP.max)
                V.tensor_tensor(out=tyv, in0=tyv, in1=yconst(1, ch), op=OP.min)

                WXs, WYs = {}, {}
                for d in (-1, 0, 1):
                    wd = pmap.tile([32, 2, N1], F32, tag=f"wd{d}",
                                   name=f"wd{d}_{ch}")
                    db = dbias[:, d + 1 : d + 2]
                    A_.activation(out=wd[:], in_=T2[:], func=AF.Abs, bias=db)
                    A_.activation(out=wd[:], in_=wd[:], func=AF.Relu,
                                  scale=-1.0, bias=dbias[:, 3:4])
                    if d != 0:
                        mi = 2 if d == -1 else 3
                        wdx = wd[:, 0, :].rearrange("p (r w) -> p r w", w=64)
                        wdy = wd[:, 1, :].rearrange("p (r w) -> p r w", w=64)
                        V.tensor_tensor(out=wdx, in0=wdx, in1=xconst(mi),
                                        op=OP.mult)
                        V.tensor_tensor(out=wdy, in0=wdy, in1=yconst(mi, ch),
                                        op=OP.mult)
                    WXs[d], WYs[d] = wd[:, 0, :], wd[:, 1, :]

                # ---------------- S maps + sim + softmax ----------------
                sim_ps = psp.tile([32, N1], F32, tag="sim", name=f"sim{ch}")
                for t in range(9):
                    dy, dx = TAPS[t]
                    o_t = 65 + 64 * dy + dx
                    M = pm.tile([128, 2, N1], F32, tag="M", name=f"M{ch}{t}")
                    V.tensor_tensor(out=M[:], in0=q[:],
                                    in1=AT[:, :, o_t : o_t + N1], op=OP.mult)
                    S_t = pmap.tile([32, N1], F32, tag="sp", bufs=4,
                                    name=f"S{ch}{t}")
                    for j in range(2):
                        sl = slice(512 * j, 512 * (j + 1))
                        s_ps = psp.tile([128, 512], F32, tag="ps512", bufs=2,
                                        name=f"sps{ch}{t}{j}")
                        for cb in range(2):
                            mm(s_ps[0:32, :], sel[:, cb, :], M[:, cb, sl],
                               start=(cb == 0), stop=(cb == 1))
                        A_.activation(out=S_t[:, sl], in_=s_ps[0:32, :],
                                      func=AF.Copy)
                    U_t = pmap.tile([32, N1], F32, tag="sp", bufs=4,
                                    name=f"U{ch}{t}")
                    V.tensor_tensor(out=U_t[:], in0=WXs[dx][:], in1=S_t[:],
                                    op=OP.mult)
                    P_t = pmap.tile([32, N1], F32, tag="sp", bufs=4,
                                    name=f"P{ch}{t}")
                    V.tensor_tensor(out=P_t[:], in0=WYs[dy][:], in1=U_t[:],
                                    op=OP.mult)
                    for j in range(2):
                        sl = slice(512 * j, 512 * (j + 1))
                        mm(sim_ps[:, sl], i32, P_t[:, sl],
                           start=(t == 0), stop=(t == 8))

                E = pmap.tile([32, N1], F32, tag="sm", bufs=3, name=f"E{ch}")
                A_.activation(out=E[:], in_=sim_ps[:], func=AF.Exp,
                              bias=dbias[:, 1:2], scale=0.125)
                Rr = pmap.tile([32, N1], F32, tag="sm", bufs=3, name=f"R{ch}")
                for j in range(2):
                    sl = slice(512 * j, 512 * (j + 1))
                    d_ps = psp.tile([128, 512], F32, tag="ps512", bufs=2,
                                    name=f"dps{ch}{j}")
                    mm(d_ps[0:32, :], hs, E[:, sl], start=True, stop=True)
                    V.reciprocal(out=Rr[:, sl], in_=d_ps[0:32, :])
                Ff = pmap.tile([32, N1], F32, tag="sm", bufs=3, name=f"F{ch}")
                V.tensor_tensor(out=Ff[:], in0=E[:], in1=Rr[:], op=OP.mult)
                WKt = pmap.tile([32, N1], F32, tag="WK", name=f"WK{ch}")
                for j in range(2):
                    sl = slice(512 * j, 512 * (j + 1))
                    wk_ps = psp.tile([128, 512], F32, tag="ps512", bufs=2,
                                     name=f"wkps{ch}{j}")
                    mm(wk_ps[0:32, :], avg4, Ff[:, sl], start=True, stop=True)
                    A_.activation(out=WKt[:, sl], in_=wk_ps[0:32, :], func=AF.Copy)

                # ---------------- G maps + final combine ----------------
                Vx = {}
                for d in (-1, 0, 1):
                    v = pmap.tile([32, N1], F32, tag=f"v{d}", name=f"v{d}_{ch}")
                    V.tensor_tensor(out=v[:], in0=WXs[d][:], in1=WKt[:],
                                    op=OP.mult)
                    Vx[d] = v

                fin = [psp.tile([128, N1], F32, tag="fin", bufs=2,
                                name=f"fin{ch}{_ob}") for _ob in range(2)]
                for t in range(9):
                    dy, dx = TAPS[t]
                    o_t = 65 + 64 * dy + dx
                    Q_t = pmap.tile([32, N1], F32, tag="qgb", bufs=4,
                                    name=f"Q{ch}{t}")
                    V.tensor_tensor(out=Q_t[:], in0=WYs[dy][:], in1=Vx[dx][:],
                                    op=OP.mult)
                    Gb = pmap.tile([128, N1], F32, tag="qgb", bufs=4,
                                   name=f"Gb{ch}{t}")
                    for j in range(2):
                        sl = slice(512 * j, 512 * (j + 1))
                        gb_ps = psp.tile([128, 512], F32, tag="ps512", bufs=2,
                                         name=f"gbps{ch}{t}{j}")
                        mm(gb_ps[:], qb, Q_t[:, sl], start=True, stop=True)
                        A_.activation(out=Gb[:, sl], in_=gb_ps[:], func=AF.Copy)
                    for ob in range(2):
                        Fv = pm.tile([128, N1], F32, tag="Fv", bufs=2,
                                     name=f"Fv{ch}{t}{ob}")
                        V.tensor_tensor(out=Fv[:], in0=Gb[:],
                                        in1=AoT[:, ob, o_t : o_t + N1], op=OP.mult)
                        for j in range(2):
                            sl = slice(512 * j, 512 * (j + 1))
                            mm(fin[ob][:, sl], i128, Fv[:, sl],
                               start=(t == 0), stop=(t == 8))

                out_sb = pbig.tile([128, 2, N1], F32, tag="osb", name=f"osb{ch}")
                for ob in range(2):
                    A_.activation(out=out_sb[:, ob, :], in_=fin[ob][:],
                                  func=AF.Copy)
                nc.gpsimd.dma_start(out=y_d[:, :, N1 * ch : N1 * (ch + 1)],
                                     in_=out_sb[:])

    split_waits(nc)
    return nc


# ============================ host-side prep ===============================

def _consts():
    perm = [2 * k for k in range(K)] + [2 * k + 1 for k in range(K)]

    sel = np.zeros((128, 2, 32), np.float32)
    for cb in range(2):
        for p in range(128):
            h = (128 * cb + p) // 64
            for j in range(32):
                if j % 4 == h:
                    sel[p, cb, j] = 1.0

    sel16 = np.zeros((16, 64), np.float32)
    for j in range(32):
        sel16[j // 4, j] = 1.0           # tx: channel k
        sel16[8 + j // 4, 32 + j] = 1.0  # ty: channel 8+k

    i32 = np.eye(32, dtype=np.float32)
    hs = np.zeros((32, 32), np.float32)
    avg4 = np.zeros((32, 32), np.float32)
    for i in range(32):
        for j in range(32):
            if i % 4 == j % 4:
                hs[i, j] = 1.0
            if i // 4 == j // 4:
                avg4[i, j] = 0.25
    qb = np.full((32, 128), 0.25, np.float32)
    i128 = np.eye(128, dtype=np.float32)
    dbias = np.zeros((32, 4), np.float32)
    dbias[:, 0], dbias[:, 2], dbias[:, 3] = 1.0, -1.0, 1.0
    return perm, sel, sel16, i32, hs, avg4, qb, i128, dbias


def _per_core_consts(h0):
    # x consts [4, 64]: lox, hix, mxm, mxp;  y consts [4, 32]: loy, hiy, mym, myp
    w = np.arange(W, dtype=np.float32)
    g = h0 + np.arange(ROWS, dtype=np.float32)
    xc = np.stack([-0.5 - w, 63.5 - w,
                   (w != 0).astype(np.float32),
                   (w != W - 1).astype(np.float32)])
    yc = np.stack([-0.5 - g, 63.5 - g,
                   (g != 0).astype(np.float32),
                   (g != H - 1).astype(np.float32)])
    return xc.astype(np.float32), yc.astype(np.float32)


def _prep_inputs(x_main, x_aux, offset_w, offset_b, q_w, k_w, out_w):
    perm, sel, sel16, i32, hs, avg4, qb, i128, dbias = _consts()

    def wT(wmat):
        # [128, 2, 2, 128]: lhsT[cin_local, cb, ob, o_local] = w[o, cin]
        r = np.zeros((128, 2, 2, 128), np.float32)
        for cb in range(2):
            for ob in range(2):
                r[:, cb, ob, :] = wmat[128 * ob : 128 * (ob + 1),
                                       128 * cb : 128 * (cb + 1)].T
        return r

    wperm = offset_w[perm]           # [16, C, 3, 3]
    bperm = offset_b[perm].astype(np.float32)
    cwT = np.zeros((128, 9, 2, 16), np.float32)
    for t, (dy, dx) in enumerate(TAPS):
        for cb in range(2):
            cwT[:, t, cb, :] = wperm[:, 128 * cb : 128 * (cb + 1),
                                     dy + 1, dx + 1].T

    wcat = np.zeros((128, 2016), np.float32)
    wcat[:, 0:512] = wT(q_w).reshape(128, 512)
    wcat[:, 512:1024] = wT(k_w).reshape(128, 512)
    wcat[:, 1024:1536] = wT(out_w).reshape(128, 512)
    wcat[:, 1536:1824] = cwT.reshape(128, 288)
    wcat[:, 1824:1888] = sel.reshape(128, 64)
    wcat[:, 1888:2016] = i128

    mcat0 = np.zeros((32, 677), np.float32)
    mcat0[:, 0:32] = i32
    mcat0[:, 32:64] = hs
    mcat0[:, 64:96] = avg4
    mcat0[:, 96:224] = qb
    mcat0[0:16, 224:288] = sel16
    mcat0[0:16, 288] = bperm
    mcat0[:, 289:293] = dbias

    in_maps = []
    for core in range(NCORES):
        b, half = core // 2, core % 2
        h0 = ROWS * half
        xm = np.zeros((128, 2, XM_W), np.float32)
        xa = np.zeros((128, 2, XA_W), np.float32)
        for r in range(HR):
            g = h0 - 1 + r
            if 0 <= g < H:
                for cb in range(2):
                    xm[:, cb, 2 + 66 * r : 2 + 66 * r + 64] = \
                        x_main[b, 128 * cb : 128 * (cb + 1), g, :]
                    xa[:, cb, 64 * r : 64 * r + 64] = \
                        x_aux[b, 128 * cb : 128 * (cb + 1), g, :]
        xc, yc = _per_core_consts(h0)
        mcat = mcat0.copy()
        for i in range(4):
            mcat[:, 293 + 64 * i : 357 + 64 * i] = xc[i][None, :]
            mcat[:, 549 + 32 * i : 581 + 32 * i] = yc[i][None, :]
        in_maps.append(dict(xm=xm, xa=xa, wcat=wcat, mcat=mcat))
    return in_maps


def kernel(**inputs):
    inputs = {k: np.asarray(v, dtype=np.float32) for k, v in inputs.items()}
    if "nc" not in _CACHE:
        _CACHE["nc"] = _build_program()
    nc = _CACHE["nc"]
    in_maps = _prep_inputs(
        inputs["x_main"], inputs["x_aux"], inputs["offset_w"],
        inputs["offset_b"], inputs["q_w"], inputs["k_w"], inputs["out_w"])
    res = run_bass_kernel_spmd(nc, in_maps, list(range(NCORES))).results

    out = np.zeros((B, C, H, W), np.float32)
    for core in range(NCORES):
        b, half = core // 2, core % 2
        y = res[core]["y"]  # [128, 2, 2048]
        for ob in range(2):
            out[b, 128 * ob : 128 * (ob + 1),
                ROWS * half : ROWS * (half + 1), :] = \
                y[:, ob, :].reshape(128, ROWS, W)
    return out
